# revision 13
# baseline (speedup 1.0000x reference)
"""Trainium2 Bass kernel for nn_AAttn (area-attention block).

Reference computation (per batch image [C=256, 64, 64]):
  qkv  = BN(1x1 conv, 3C)          -> split per head h: q,k,v (hd=32)
  area attention over 4 areas of 1024 px each (16 rows x 64 cols)
  o    = softmax(q^T k / sqrt(hd)) @ v
  pe   = BN(7x7 depthwise conv(v_map))
  out  = BN(1x1 conv(o + pe))

Sharding: fully data-parallel. 8 cores x (1 batch-half = 2 areas = 32 rows).
Each core gets a 38-row halo slab of x; everything else is computed locally
(halo of v for the depthwise conv is recomputed from the x halo). No
collectives.

Depthwise conv: all 49 taps run on PE as PACKED matmuls. DMA engines
replicate shifted copies of v into "v4" tiles whose 128 partitions hold
(tap-slot, 32-channel-block) pairs; a block-diagonal [128,32] lhsT then
computes the weighted sum of 4 (or 3) vertically-stacked taps per matmul
at the PE's M<=32 fast rate (~60ns per 512-px chunk). Patterns P4 =
dy in {-3,-1,1,3} and P3 = dy in {-2,0,2} cover all 7 rows; the 7 dx
shifts are free column offsets into the same v4 copy. 56 matmuls per
512-px chunk replace 49 full-width diagonal matmuls (3.4us vs 12.7us).

Softmax exps are split Scalar/DVE/GPSIMD per unit-group; DVE and GPSIMD
use a one-instruction Schraudolph bit-trick (fp32->int16 mult-add,
bitcast bf16).
"""

import os
import sys

os.environ.setdefault("MYCRO_LOCAL_CACHE", "1")
if "/opt/trn_rl_repo" not in sys.path:
    sys.path.insert(0, "/opt/trn_rl_repo")

from contextlib import ExitStack

import ml_dtypes
import numpy as np

import concourse.bass as bass
import concourse.bacc as bacc
import concourse.tile as tile
from concourse import mybir
from concourse.bass_utils import run_bass_kernel_spmd


def _install_ntff_hook_shim():
    """The agent image's antenv lacks axon_hooks; recreate it so
    run_bass_kernel_spmd(trace=True) can NTFF-profile via the axon .so."""
    import types
    try:
        from antenv.axon_hooks import get_axon_ntff_profile_hook  # noqa: F401
        return  # real module exists
    except ImportError:
        pass
    try:
        from trn_agent_boot.trn_boot import _ntff_profile_via_ctypes
        hook = _ntff_profile_via_ctypes("/opt/axon/libaxon_pjrt.so")
    except Exception:
        hook = None
    mod = types.ModuleType("antenv.axon_hooks")
    _state = {"hook": hook}
    mod.get_axon_ntff_profile_hook = lambda: _state["hook"]
    mod.set_axon_ntff_profile_hook = lambda h: _state.update(hook=h)
    sys.modules["antenv.axon_hooks"] = mod
    import antenv
    antenv.axon_hooks = mod


_install_ntff_hook_shim()

F32 = mybir.dt.float32
BF16 = mybir.dt.bfloat16
I16 = mybir.dt.int16
BF16NP = ml_dtypes.bfloat16

B, C, H, W = 4, 256, 64, 64
NH, HD, AREA = 8, 32, 4
EPS = 1e-5
NCORES = 8

CORE_ROWS = 32          # image rows per core
HALO = 3                # 7x7 conv halo
SLAB_ROWS = CORE_ROWS + 2 * HALO   # 38
PX = SLAB_ROWS * W      # 2432 slab pixels
CPX = CORE_ROWS * W     # 2048 core pixels
PXOFF = HALO * W        # 192: slab px offset of core region
NA = 1024               # pixels per area

VROW = W + 6            # v row pitch: 3 zero cols each side
VP = VROW * SLAB_ROWS   # 2660
SEC = 8 * VROW          # v4 section stride (560): 8 conv rows incl pads

LAST_EXEC_NS = [None]
LAST_RESULTS = [None]

# packed-conv patterns: vertical tap stacks; dx handled by column offsets
PATTERNS = [(-3, -1, 1, 3), (-2, 0, 2)]
NSEC = 4 * len(PATTERNS)             # (block, pattern) sections per chunk

# softmax-exp engine per unit index mod 16: 'S' = ScalarE Exp, 'D' = DVE
# Schraudolph bit-trick. (GPSIMD cannot read PSUM, so no exps there.)
# 9:7 split keeps both engines ~equally loaded incl. their misc work.
DVE_UNITS = {1, 3, 5, 7, 9, 11, 13}
NSCHAIN = 5   # independent single-bank psum chains for S tiles

# Schraudolph bf16-exp constants: bits = round(x * 128/ln2 + c2)
EXP_C1 = 128.0 / float(np.log(2.0))
EXP_C2 = 127.0 * 128.0 - 128.0 * 0.043


def _build_graph():
    nc = bacc.Bacc()

    x_ext = nc.declare_dram_parameter("x", [C, PX], BF16, isOutput=False)
    vmask_ext = nc.declare_dram_parameter("vmask", [1, PX], BF16, isOutput=False)
    bvrow_ext = nc.declare_dram_parameter("bvrow", [1, C], BF16, isOutput=False)
    wqkv_ext = nc.declare_dram_parameter("wqkv", [C, 3 * C], BF16, isOutput=False)
    bqkv_ext = nc.declare_dram_parameter("bqkv", [3 * C, 1], F32, isOutput=False)
    w4_ext = nc.declare_dram_parameter("w4", [128, 2 * 2 * 7 * 4 * 32], BF16,
                                       isOutput=False)
    wproj_ext = nc.declare_dram_parameter("wproj", [C, C], BF16, isOutput=False)
    btot_ext = nc.declare_dram_parameter("btot", [C, 1], F32, isOutput=False)
    out_ext = nc.declare_dram_parameter("out", [C, CPX], F32, isOutput=True)

    with tile.TileContext(nc) as tc, ExitStack() as ctx:
        persist = ctx.enter_context(tc.tile_pool(name="persist", bufs=1))
        e_pool = ctx.enter_context(tc.tile_pool(name="epool", bufs=8))
        wk_pool = ctx.enter_context(tc.tile_pool(name="wkpool", bufs=2))
        v4_pool = ctx.enter_context(tc.tile_pool(name="v4pool", bufs=4))
        mm_ctx = tc.tile_pool(name="mmps", bufs=3, space="PSUM")
        mm_ps = mm_ctx.__enter__()

        def ptile(shape, dtype, name):
            return persist.tile(shape, dtype, name=name, tag=name)

        # ---------------- persistent SBUF tensors ----------------
        wp_t = [ptile([128, C], BF16, name=f"wp{k}") for k in range(2)]
        btot_t = [ptile([128, 1], F32, name=f"btot{m}") for m in range(2)]
        w4_t = ptile([128, 2 * 2 * 7 * 4 * 32], BF16, name="w4")
        ones_t = ptile([128, 32], BF16, name="ones")

        q_sb = [ptile([128, CPX], BF16, name=f"q{h}") for h in range(2)]
        k_sb = [ptile([128, CPX], BF16, name=f"k{h}") for h in range(2)]
        v_sb = [ptile([128, VP], BF16, name=f"v{cti}") for cti in range(2)]
        vt_sb = [ptile([128, 2048], BF16, name=f"vt{a}") for a in range(2)]
        onorm_sb = [ptile([128, CPX], BF16, name=f"onorm{h}") for h in range(2)]
        pin_sb = [ptile([128, CPX], BF16, name=f"pin{cti}") for cti in range(2)]
        out_sb = [ptile([128, CPX], F32, name=f"outsb{cti}") for cti in range(2)]

        # early (released before attention): x, qkv weights, mask
        x_t = [ptile([128, PX], BF16, name=f"x{k}") for k in range(2)]
        wq_t = [ptile([128, 3 * C], BF16, name=f"wq{k}") for k in range(2)]
        bias_t = [ptile([128, 1], F32, name=f"bias{m}") for m in range(4)]
        maskr_t = ptile([1, PX], BF16, name="maskr")
        bvrow_t = ptile([1, C], BF16, name="bvrow")

        # ---------------- input DMAs (priority order) ----------------
        for k in range(2):
            nc.sync.dma_start(x_t[k][:], x_ext[128 * k:128 * (k + 1), :])
            nc.sync.dma_start(wq_t[k][:], wqkv_ext[128 * k:128 * (k + 1), :])
        for m in range(4):
            nc.sync.dma_start(bias_t[m][:], bqkv_ext[128 * m:128 * (m + 1), :])
        nc.sync.dma_start(maskr_t[:], vmask_ext[:])
        nc.sync.dma_start(bvrow_t[:], bvrow_ext[:])
        nc.sync.dma_start(w4_t[:], w4_ext[:])
        for k in range(2):
            nc.sync.dma_start(wp_t[k][:], wproj_ext[128 * k:128 * (k + 1), :])
            nc.sync.dma_start(btot_t[k][:], btot_ext[128 * k:128 * (k + 1), :])
        nc.vector.memset(ones_t[:], 1.0)
        for k in range(2):
            # zero everything; evacs fill the 64-wide data blocks of each row
            nc.gpsimd.memset(v_sb[k][:], 0.0)

        # ---------------- qkv 1x1 conv (matmul) + BN ----------------
        # Only head-set 0 Q/K before attention starts; biases folded into
        # the evacuation instructions (ScalarE Identity+bias / DVE TSP-add).
        for mc in (0, 2):
            pcs = [(i * 1024, 1024) for i in range(2)]
            for ti_, (pco, pcn) in enumerate(pcs):
                ps = mm_ps.tile([128, 1024], F32, tag="mm")
                for half in range(0, pcn, 512):
                    hn = min(512, pcn - half)
                    for kc in range(2):
                        nc.tensor.matmul(
                            ps[:, half:half + hn],
                            lhsT=wq_t[kc][:, 128 * mc:128 * (mc + 1)],
                            rhs=x_t[kc][:, PXOFF + pco + half:
                                        PXOFF + pco + half + hn],
                            start=(kc == 0), stop=(kc == 1),
                        )
                dst = q_sb[0] if mc == 0 else k_sb[0]
                if ti_ % 2 == 0:
                    nc.scalar.activation(
                        dst[:, pco:pco + pcn], ps[:, :pcn],
                        mybir.ActivationFunctionType.Identity,
                        bias=bias_t[mc][:, 0:1])
                else:
                    nc.vector.tensor_scalar_add(
                        dst[:, pco:pco + pcn], ps[:, :pcn],
                        bias_t[mc][:, 0:1])

        mm_ctx.__exit__(None, None, None)
        s_ps_ctx = tc.tile_pool(name="sps", bufs=1, space="PSUM")
        s_ps_pool = s_ps_ctx.__enter__()
        od_ctx = tc.tile_pool(name="odps", bufs=3, space="PSUM")
        od_pool = od_ctx.__enter__()

        # ------------- attention (phase-split) + packed conv -------------
        def qkv_tail_piece(od_pool_, kind, arg):
            if kind == "qk":
                mc, pco = arg
                pcn = 512
                ps = od_pool_.tile([128, 512], F32, tag="od")
                for kc in range(2):
                    nc.tensor.matmul(
                        ps[:, :pcn],
                        lhsT=wq_t[kc][:, 128 * mc:128 * (mc + 1)],
                        rhs=x_t[kc][:, PXOFF + pco:PXOFF + pco + pcn],
                        start=(kc == 0), stop=(kc == 1),
                    )
                dst = q_sb[1] if mc == 1 else k_sb[1]
                nc.vector.tensor_scalar_add(
                    dst[:, pco:pco + pcn], ps[:, :pcn], bias_t[mc][:, 0:1])
            elif kind == "v":
                mc, pco = arg
                cti = mc - 4
                pcn = min(512, PX - pco)
                ps = od_pool_.tile([128, 512], F32, tag="od")
                for kc in range(2):
                    nc.tensor.matmul(
                        ps[:, :pcn],
                        lhsT=wq_t[kc][:, 128 * mc:128 * (mc + 1)],
                        rhs=x_t[kc][:, pco:pco + pcn],
                        start=(kc == 0), stop=False,
                    )
                # masked bias as a rank-1 update: psum += b_v ⊗ mask
                nc.tensor.matmul(
                    ps[:, :pcn],
                    lhsT=bvrow_t[:, 128 * cti:128 * (cti + 1)],
                    rhs=maskr_t[:, pco:pco + pcn],
                    start=False, stop=True,
                )
                r0, nr = pco // W, (pcn + W - 1) // W
                v70 = v_sb[cti][:].rearrange("p (r c) -> p r c", c=VROW)
                dst = v70[:, r0:r0 + nr, 3:3 + W]
                if (pco // 512) % 2 == 0:
                    nc.scalar.activation(dst, ps[:, :pcn],
                                         mybir.ActivationFunctionType.Copy)
                else:
                    nc.vector.tensor_copy(dst, ps[:, :pcn])
            else:  # vT
                a, g = arg
                ps = od_pool_.tile([128, 512], F32, tag="od")
                for jj in range(2):
                    j = 2 * g + jj
                    pxo = PXOFF + NA * a + 128 * j
                    for kc in range(2):
                        nc.tensor.matmul(
                            ps[:, 256 * jj:256 * (jj + 1)],
                            lhsT=x_t[kc][:, pxo:pxo + 128],
                            rhs=wq_t[kc][:, 2 * C:3 * C],
                            start=(kc == 0), stop=(kc == 1),
                        )
                dst = vt_sb[a][:, 512 * g:512 * (g + 1)]
                if g % 2 == 0:
                    nc.scalar.activation(dst, ps[:, 0:512],
                                         mybir.ActivationFunctionType.Copy)
                else:
                    nc.vector.tensor_copy(dst, ps[:, 0:512])

        def v4_copies(ci):
            """DMA-replicate shifted v windows for conv chunk ci into a v4
            tile: section (pat, b) holds, at partitions 32t..32t+32, block
            b's channels shifted for tap-row dy_t; returns the tile."""
            cti, c = ci // 4, ci % 4
            v4 = v4_pool.tile([128, NSEC * SEC], BF16, tag="v4",
                              name=f"v4_{ci}")
            for pi, pat in enumerate(PATTERNS):
                for b in range(4):
                    sec = (pi * 4 + b) * SEC
                    for t, dy in enumerate(pat):
                        src = (HALO + dy + 8 * c) * VROW
                        nc.sync.dma_start(
                            v4[32 * t:32 * (t + 1), sec:sec + SEC],
                            v_sb[cti][32 * b:32 * (b + 1), src:src + SEC])
            return v4

        def conv_chunk_mms(ci, v4, ps):
            cti, c = ci // 4, ci % 4
            for dxi in range(7):
                for pi, pat in enumerate(PATTERNS):
                    kp = 32 * len(pat)
                    for b in range(4):
                        sec = (pi * 4 + b) * SEC
                        v4r = v4[0:kp, sec:sec + SEC].rearrange(
                            "p (r c) -> p r c", c=VROW)
                        wcol = 32 * (((cti * 2 + pi) * 7 + dxi) * 4 + b)
                        nc.tensor.matmul(
                            ps[32 * b:32 * (b + 1), :],
                            lhsT=w4_t[0:kp, wcol:wcol + 32],
                            rhs=v4r[:, 0:8, dxi:dxi + W],
                            start=(dxi == 0 and pi == 0),
                            stop=(dxi == 6 and pi == len(PATTERNS) - 1),
                            skip_group_check=True,
                            tile_position=(0, 32 * b),
                        )

        def conv_chunk_evac(ci, ps):
            # pin = conv_psum + attention-output chunk (same 512 cols)
            cti, c = ci // 4, ci % 4
            nc.vector.tensor_tensor(
                pin_sb[cti][:, 512 * c:512 * (c + 1)], ps,
                onorm_sb[cti][:, 512 * c:512 * (c + 1)],
                mybir.AluOpType.add)

        # PE filler schedule: vT0+v0 early (conv chunk 0 copies need v0 at
        # i0), v1/vT1/Q1K1 spread over i1-i3 (Q1/K1 first read at i4).
        tail_work = (
            [("vT", (0, g)) for g in range(4)]
            + [("v", (4, 512 * t)) for t in range(5)]
            + [("vT", (1, g)) for g in range(4)]
            + [("v", (5, 512 * t)) for t in range(5)]
            + [("qk", (1, 512 * t)) for t in range(4)]
            + [("qk", (3, 512 * t)) for t in range(4)]
        )
        tail_sched = {0: 9, 1: 8, 2: 5, 3: 4}
        # v4 replication DMAs prefetched 2-3 iters ahead of consumption
        v4_sched = {0: [0, 1, 2], 1: [3], 3: [4, 5], 4: [6], 5: [7]}

        def emit_proj(pc):
            for mc in range(2):
                ps = od_pool.tile([128, 512], F32, tag="od")
                for kc in range(2):
                    nc.tensor.matmul(
                        ps[:],
                        lhsT=wp_t[kc][:, 128 * mc:128 * (mc + 1)],
                        rhs=pin_sb[kc][:, 512 * pc:512 * (pc + 1)],
                        start=(kc == 0), stop=(kc == 1),
                    )
                nc.scalar.activation(
                    out_sb[mc][:, 512 * pc:512 * (pc + 1)], ps[:],
                    mybir.ActivationFunctionType.Identity,
                    bias=btot_t[mc][:, 0:1])
                nc.sync.dma_start(
                    out_ext[128 * mc:128 * (mc + 1), 512 * pc:512 * (pc + 1)],
                    out_sb[mc][:, 512 * pc:512 * (pc + 1)])

        v4_tiles = {}
        its = [(hs, a, nu) for hs in range(2) for a in range(2) for nu in range(2)]
        for inum, (hs, a, nu) in enumerate(its):
            no = NA * a + 512 * nu   # n offset in core px
            units = [(j, hp) for j in range(8) for hp in range(4)]
            # ---- phase A: S + exp -> E tiles ----
            # Single-unit groups across NSCHAIN independent one-bank psum
            # chains so exp latency never stalls the next S matmul; exps
            # alternate ScalarE / DVE per DVE_UNITS.
            e_tiles = []
            for u, (j, hp) in enumerate(units):
                s_ps = s_ps_pool.tile([128, 512], F32, tag=f"s{u % NSCHAIN}")
                nc.tensor.matmul(
                    s_ps[:],
                    lhsT=k_sb[hs][32 * hp:32 * (hp + 1),
                                  NA * a + 128 * j:NA * a + 128 * (j + 1)],
                    rhs=q_sb[hs][32 * hp:32 * (hp + 1), no:no + 512],
                    start=True, stop=True,
                    tile_position=(32 * hp, 0),
                )
                on_dve = (u % 16) in DVE_UNITS
                e_t = e_pool.tile([128, 512], BF16,
                                  tag="ed" if on_dve else "e",
                                  bufs=8 if on_dve else 10)
                if on_dve:
                    nc.vector.tensor_scalar(
                        e_t[:].bitcast(I16), s_ps[:],
                        EXP_C1, EXP_C2,
                        mybir.AluOpType.mult, mybir.AluOpType.add)
                else:
                    nc.scalar.activation(
                        e_t[:], s_ps[:],
                        mybir.ActivationFunctionType.Exp)
                e_tiles.append(((j, hp), e_t))
            # PE filler between S production and O/den consumption: qkv
            # tail pieces + this iteration's conv chunk (contiguous run).
            nwork = tail_sched.get(inum, 0)
            for _ in range(nwork):
                if tail_work:
                    qkv_tail_piece(od_pool, *tail_work.pop(0))
            for ci in v4_sched.get(inum, []):
                v4_tiles[ci] = v4_copies(ci)
            if inum > 0:
                ci = inum - 1
                cv_ps = od_pool.tile([128, 512], F32, tag="od",
                                     name=f"cv{ci}")
                conv_chunk_mms(ci, v4_tiles.pop(ci), cv_ps[:])
                conv_chunk_evac(ci, cv_ps[:])
                # proj chunk pc as soon as pin[1][pc] completes (evac ci=4+pc)
                if inum >= 5:
                    emit_proj(inum - 5)
            # ---- phase B: dense O + den burst ----
            o_ps = od_pool.tile([128, 512], F32, tag="od")
            den_ps = od_pool.tile([128, 512], F32, tag="od")
            for (j, hp), e_t in e_tiles:
                first, last = (j == 0), (j == 7)
                nc.tensor.matmul(
                    o_ps[32 * hp:32 * (hp + 1), :],
                    lhsT=vt_sb[a][:, 256 * j + 32 * (4 * hs + hp):
                                   256 * j + 32 * (4 * hs + hp + 1)],
                    rhs=e_t[:],
                    start=first, stop=last,
                    skip_group_check=True,
                    tile_position=(0, 32 * hp),
                )
                nc.tensor.matmul(
                    den_ps[32 * hp:32 * (hp + 1), :],
                    lhsT=ones_t[:, 0:32],
                    rhs=e_t[:],
                    start=first, stop=last,
                    skip_group_check=True,
                    tile_position=(0, 32 * hp),
                )
            if inum == 7:
                # chunk 7's matmuls run while the last exps/O drain; its
                # evacuation (needs onorm chunk 7) happens in the tail.
                ps7w = od_pool.tile([128, 512], F32, tag="od", name="cv7")
                ps7 = ps7w[:]
                conv_chunk_mms(7, v4_tiles.pop(7), ps7)
            rd32 = wk_pool.tile([128, 512], F32, tag="rd32")
            nc.vector.reciprocal_approx_fast(rd32[:], den_ps[:])
            nc.vector.tensor_mul(
                onorm_sb[hs][:, no:no + 512], o_ps[:], rd32[:])

        # ---------------- tail: last conv chunk + proj ----------------
        conv_chunk_evac(7, ps7)
        emit_proj(3)

        od_ctx.__exit__(None, None, None)
        s_ps_ctx.__exit__(None, None, None)

    nc.finalize()
    return nc


_GRAPH = None


def kernel(**inputs):
    global _GRAPH
    inputs = {k: np.asarray(v, np.float32) for k, v in inputs.items()}
    x = inputs["x"]

    def fold(g, b, m, v):
        inv = g / np.sqrt(v + EPS)
        return inv, b - m * inv

    sq, bq = fold(inputs["qkv_g"], inputs["qkv_b"], inputs["qkv_m"], inputs["qkv_v"])
    spe, bpe = fold(inputs["pe_g"], inputs["pe_b"], inputs["pe_m"], inputs["pe_v"])
    sp, bp = fold(inputs["proj_g"], inputs["proj_b"], inputs["proj_m"], inputs["proj_v"])

    wqkv = np.asarray(inputs["qkv_w"], np.float32)[:, :, 0, 0] * sq[:, None]  # [768,256]
    bqkv = np.asarray(bq, np.float32)
    # permute rows to head-major [Q(256); K(256); V(256)]
    perm = np.empty(3 * C, np.int64)
    for h in range(NH):
        for t in range(3):
            for d in range(HD):
                perm[t * C + HD * h + d] = 3 * HD * h + HD * t + d
    wqkv = wqkv[perm]
    bqkv = bqkv[perm]
    scale = HD ** -0.5
    wqkv[:C] *= scale
    bqkv[:C] *= scale
    b_v = bqkv[2 * C:].copy()

    wpe = np.asarray(inputs["pe_w"], np.float32)[:, 0].reshape(C, 49) * spe[:, None]
    wproj = np.asarray(inputs["proj_w"], np.float32)[:, :, 0, 0] * sp[:, None]
    btot = bp + wproj @ (b_v + bpe)

    wqkv_T = np.ascontiguousarray(wqkv.T).astype(BF16NP)          # [256, 768]
    wproj_T = np.ascontiguousarray(wproj.T).astype(BF16NP)        # [256, 256]
    bqkv_c = np.ascontiguousarray(bqkv[:, None]).astype(np.float32)
    btot_c = np.ascontiguousarray(btot[:, None]).astype(np.float32)

    # packed conv weights: w4[32t+cc, col(cti,pi,dxi,b)*32+m] =
    #   delta(cc,m) * wpe[128*cti+32*b+m, tap(dy_t, dx)]
    w4 = np.zeros((128, 2 * 2 * 7 * 4 * 32), np.float32)
    for cti in range(2):
        for pi, pat in enumerate(PATTERNS):
            for dxi in range(7):
                dx = dxi - 3
                for b in range(4):
                    col = 32 * (((cti * 2 + pi) * 7 + dxi) * 4 + b)
                    for t, dy in enumerate(pat):
                        tap = (dy + 3) * 7 + (dx + 3)
                        ch0 = 128 * cti + 32 * b
                        for m in range(32):
                            w4[32 * t + m, col + m] = wpe[ch0 + m, tap]
    w4 = np.ascontiguousarray(w4).astype(BF16NP)

    xp = np.zeros((B, C, H + 2 * HALO, W), np.float32)
    xp[:, :, HALO:HALO + H] = x

    in_maps = []
    for i in range(NCORES):
        b, r0 = i // 2, 32 * (i % 2)
        slab = xp[b, :, r0:r0 + SLAB_ROWS, :].reshape(C, PX)
        vmask = np.zeros((1, PX), np.float32)
        vr = np.zeros(SLAB_ROWS, np.float32)
        if i % 2 == 0:
            vr[HALO:] = 1.0          # slab rows 0-2 are outside the image
        else:
            vr[:SLAB_ROWS - HALO] = 1.0
        vmask[0] = np.repeat(vr, W)
        in_maps.append({
            "x": slab.astype(BF16NP),
            "vmask": vmask.astype(BF16NP),
            "bvrow": np.ascontiguousarray(b_v[None, :]).astype(BF16NP),
            "w4": w4,
            "wqkv": wqkv_T,
            "bqkv": bqkv_c,
            "wproj": wproj_T,
            "btot": btot_c,
        })

    if _GRAPH is None:
        _GRAPH = _build_graph()

    trace = os.environ.get("BASS_KERNEL_TRACE") == "1"
    res = run_bass_kernel_spmd(_GRAPH, in_maps, list(range(NCORES)), trace=trace)
    LAST_EXEC_NS[0] = res.exec_time_ns
    LAST_RESULTS[0] = res.results[0]

    out = np.empty((B, C, H, W), np.float32)
    for i in range(NCORES):
        b, r0 = i // 2, 32 * (i % 2)
        out[b, :, r0:r0 + 32, :] = np.asarray(
            res.results[i]["out"], np.float32).reshape(C, 32, W)
    return out


# revision 16
# speedup vs baseline: 1.2091x; 1.2091x over previous
"""Trainium2 Bass kernel for nn_AAttn (area-attention block).

Reference computation (per batch image [C=256, 64, 64]):
  qkv  = BN(1x1 conv, 3C)          -> split per head h: q,k,v (hd=32)
  area attention over 4 areas of 1024 px each (16 rows x 64 cols)
  o    = softmax(q^T k / sqrt(hd)) @ v
  pe   = BN(7x7 depthwise conv(v_map))
  out  = BN(1x1 conv(o + pe))

Sharding: fully data-parallel. 8 cores x (1 batch-half = 2 areas = 32 rows).
Each core gets a 38-row halo slab of x; everything else is computed locally
(halo of v for the depthwise conv is recomputed from the x halo). No
collectives.

Depthwise conv: all 49 taps run on PE as PACKED matmuls. DMA engines
replicate shifted copies of v into "v4" tiles whose 128 partitions hold
(tap-slot, 32-channel-block) pairs; a block-diagonal [128,32] lhsT then
computes the weighted sum of 4 (or 3) vertically-stacked taps per matmul
at the PE's M<=32 fast rate (~60ns per 512-px chunk). Patterns P4 =
dy in {-3,-1,1,3} and P3 = dy in {-2,0,2} cover all 7 rows; the 7 dx
shifts are free column offsets into the same v4 copy. 56 matmuls per
512-px chunk replace 49 full-width diagonal matmuls (3.4us vs 12.7us).

Softmax exps are split Scalar/DVE/GPSIMD per unit-group; DVE and GPSIMD
use a one-instruction Schraudolph bit-trick (fp32->int16 mult-add,
bitcast bf16).
"""

import os
import sys

os.environ.setdefault("MYCRO_LOCAL_CACHE", "1")
if "/opt/trn_rl_repo" not in sys.path:
    sys.path.insert(0, "/opt/trn_rl_repo")

from contextlib import ExitStack

import ml_dtypes
import numpy as np

import concourse.bass as bass
import concourse.bacc as bacc
import concourse.tile as tile
from concourse import mybir
from concourse.bass_utils import run_bass_kernel_spmd


def _install_ntff_hook_shim():
    """The agent image's antenv lacks axon_hooks; recreate it so
    run_bass_kernel_spmd(trace=True) can NTFF-profile via the axon .so."""
    import types
    try:
        from antenv.axon_hooks import get_axon_ntff_profile_hook  # noqa: F401
        return  # real module exists
    except ImportError:
        pass
    try:
        from trn_agent_boot.trn_boot import _ntff_profile_via_ctypes
        hook = _ntff_profile_via_ctypes("/opt/axon/libaxon_pjrt.so")
    except Exception:
        hook = None
    mod = types.ModuleType("antenv.axon_hooks")
    _state = {"hook": hook}
    mod.get_axon_ntff_profile_hook = lambda: _state["hook"]
    mod.set_axon_ntff_profile_hook = lambda h: _state.update(hook=h)
    sys.modules["antenv.axon_hooks"] = mod
    import antenv
    antenv.axon_hooks = mod


_install_ntff_hook_shim()

F32 = mybir.dt.float32
BF16 = mybir.dt.bfloat16
I16 = mybir.dt.int16
BF16NP = ml_dtypes.bfloat16

B, C, H, W = 4, 256, 64, 64
NH, HD, AREA = 8, 32, 4
EPS = 1e-5
NCORES = 8

CORE_ROWS = 32          # image rows per core
HALO = 3                # 7x7 conv halo
SLAB_ROWS = CORE_ROWS + 2 * HALO   # 38
PX = SLAB_ROWS * W      # 2432 slab pixels
CPX = CORE_ROWS * W     # 2048 core pixels
PXOFF = HALO * W        # 192: slab px offset of core region
NA = 1024               # pixels per area

VROW = W + 6            # v row pitch: 3 zero cols each side
VP = VROW * SLAB_ROWS   # 2660
SEC = 8 * VROW          # v4 section stride (560): 8 conv rows incl pads

LAST_EXEC_NS = [None]
LAST_RESULTS = [None]

# packed-conv patterns: vertical tap stacks; dx handled by column offsets
PATTERNS = [(-3, -1, 1, 3), (-2, 0, 2)]
NSEC = 4 * len(PATTERNS)             # (block, pattern) sections per chunk

# phase-A group pattern: alternating sA (2 units -> DVE Schraudolph exp)
# and sB (3 units -> ScalarE Exp) psum chains; 2+3 banks, chain stalls
# hidden by interleaving conv/tail PE work between groups. GPSIMD cannot
# read PSUM, so no exps there.
GROUP_SIZES = [2, 3] * 6 + [2]   # 32 units -> 7 A-groups + 6 B-groups

# Schraudolph bf16-exp constants: bits = round(x * 128/ln2 + c2)
EXP_C1 = 128.0 / float(np.log(2.0))
EXP_C2 = 127.0 * 128.0 - 128.0 * 0.043


def _build_graph():
    nc = bacc.Bacc()

    x_ext = nc.declare_dram_parameter("x", [C, PX], BF16, isOutput=False)
    vmask_ext = nc.declare_dram_parameter("vmask", [1, PX], BF16, isOutput=False)
    bvrow_ext = nc.declare_dram_parameter("bvrow", [1, C], BF16, isOutput=False)
    wqkv_ext = nc.declare_dram_parameter("wqkv", [C, 3 * C], BF16, isOutput=False)
    bqkv_ext = nc.declare_dram_parameter("bqkv", [3 * C, 1], F32, isOutput=False)
    w4_ext = nc.declare_dram_parameter("w4", [128, 2 * 2 * 7 * 4 * 32], BF16,
                                       isOutput=False)
    wproj_ext = nc.declare_dram_parameter("wproj", [C, C], BF16, isOutput=False)
    btot_ext = nc.declare_dram_parameter("btot", [C, 1], F32, isOutput=False)
    out_ext = nc.declare_dram_parameter("out", [C, CPX], F32, isOutput=True)

    with tile.TileContext(nc) as tc, ExitStack() as ctx:
        persist = ctx.enter_context(tc.tile_pool(name="persist", bufs=1))
        e_pool = ctx.enter_context(tc.tile_pool(name="epool", bufs=8))
        wk_pool = ctx.enter_context(tc.tile_pool(name="wkpool", bufs=2))
        v4_pool = ctx.enter_context(tc.tile_pool(name="v4pool", bufs=4))
        mm_ctx = tc.tile_pool(name="mmps", bufs=3, space="PSUM")
        mm_ps = mm_ctx.__enter__()

        def ptile(shape, dtype, name):
            return persist.tile(shape, dtype, name=name, tag=name)

        # ---------------- persistent SBUF tensors ----------------
        wp_t = [ptile([128, C], BF16, name=f"wp{k}") for k in range(2)]
        btot_t = [ptile([128, 1], F32, name=f"btot{m}") for m in range(2)]
        w4_t = ptile([128, 2 * 2 * 7 * 4 * 32], BF16, name="w4")
        ones_t = ptile([128, 32], BF16, name="ones")

        q_sb = [ptile([128, CPX], BF16, name=f"q{h}") for h in range(2)]
        k_sb = [ptile([128, CPX], BF16, name=f"k{h}") for h in range(2)]
        v_sb = [ptile([128, VP], BF16, name=f"v{cti}") for cti in range(2)]
        vt_sb = [ptile([128, 2048], BF16, name=f"vt{a}") for a in range(2)]
        onorm_sb = [ptile([128, CPX], BF16, name=f"onorm{h}") for h in range(2)]
        pin_sb = [ptile([128, CPX], BF16, name=f"pin{cti}") for cti in range(2)]
        out_sb = [ptile([128, CPX], F32, name=f"outsb{cti}") for cti in range(2)]

        # early (released before attention): x, qkv weights, mask
        x_t = [ptile([128, PX], BF16, name=f"x{k}") for k in range(2)]
        wq_t = [ptile([128, 3 * C], BF16, name=f"wq{k}") for k in range(2)]
        bias_t = [ptile([128, 1], F32, name=f"bias{m}") for m in range(4)]
        maskr_t = ptile([1, PX], BF16, name="maskr")
        bvrow_t = ptile([1, C], BF16, name="bvrow")

        # ---------------- input DMAs (priority order) ----------------
        for k in range(2):
            nc.sync.dma_start(x_t[k][:], x_ext[128 * k:128 * (k + 1), :])
            nc.sync.dma_start(wq_t[k][:], wqkv_ext[128 * k:128 * (k + 1), :])
        for m in range(4):
            nc.sync.dma_start(bias_t[m][:], bqkv_ext[128 * m:128 * (m + 1), :])
        nc.sync.dma_start(maskr_t[:], vmask_ext[:])
        nc.sync.dma_start(bvrow_t[:], bvrow_ext[:])
        nc.sync.dma_start(w4_t[:], w4_ext[:])
        for k in range(2):
            nc.sync.dma_start(wp_t[k][:], wproj_ext[128 * k:128 * (k + 1), :])
            nc.sync.dma_start(btot_t[k][:], btot_ext[128 * k:128 * (k + 1), :])
        nc.vector.memset(ones_t[:], 1.0)
        for k in range(2):
            # zero everything; evacs fill the 64-wide data blocks of each row
            nc.gpsimd.memset(v_sb[k][:], 0.0)

        # ---------------- qkv 1x1 conv (matmul) + BN ----------------
        # Only head-set 0 Q/K before attention starts; biases folded into
        # the evacuation instructions (ScalarE Identity+bias / DVE TSP-add).
        for mc in (0, 2):
            pcs = [(i * 1024, 1024) for i in range(2)]
            for ti_, (pco, pcn) in enumerate(pcs):
                ps = mm_ps.tile([128, 1024], F32, tag="mm")
                for half in range(0, pcn, 512):
                    hn = min(512, pcn - half)
                    for kc in range(2):
                        nc.tensor.matmul(
                            ps[:, half:half + hn],
                            lhsT=wq_t[kc][:, 128 * mc:128 * (mc + 1)],
                            rhs=x_t[kc][:, PXOFF + pco + half:
                                        PXOFF + pco + half + hn],
                            start=(kc == 0), stop=(kc == 1),
                        )
                dst = q_sb[0] if mc == 0 else k_sb[0]
                if ti_ % 2 == 0:
                    nc.scalar.activation(
                        dst[:, pco:pco + pcn], ps[:, :pcn],
                        mybir.ActivationFunctionType.Identity,
                        bias=bias_t[mc][:, 0:1])
                else:
                    nc.vector.tensor_scalar_add(
                        dst[:, pco:pco + pcn], ps[:, :pcn],
                        bias_t[mc][:, 0:1])

        mm_ctx.__exit__(None, None, None)
        s_ps_ctx = tc.tile_pool(name="sps", bufs=1, space="PSUM")
        s_ps_pool = s_ps_ctx.__enter__()
        od_ctx = tc.tile_pool(name="odps", bufs=3, space="PSUM")
        od_pool = od_ctx.__enter__()

        # ------------- attention (phase-split) + packed conv -------------
        def qkv_tail_piece(od_pool_, kind, arg):
            if kind == "qk":
                mc, pco = arg
                pcn = 512
                ps = od_pool_.tile([128, 512], F32, tag="od")
                for kc in range(2):
                    nc.tensor.matmul(
                        ps[:, :pcn],
                        lhsT=wq_t[kc][:, 128 * mc:128 * (mc + 1)],
                        rhs=x_t[kc][:, PXOFF + pco:PXOFF + pco + pcn],
                        start=(kc == 0), stop=(kc == 1),
                    )
                dst = q_sb[1] if mc == 1 else k_sb[1]
                nc.vector.tensor_scalar_add(
                    dst[:, pco:pco + pcn], ps[:, :pcn], bias_t[mc][:, 0:1])
            elif kind == "v":
                mc, pco = arg
                cti = mc - 4
                pcn = min(512, PX - pco)
                ps = od_pool_.tile([128, 512], F32, tag="od")
                for kc in range(2):
                    nc.tensor.matmul(
                        ps[:, :pcn],
                        lhsT=wq_t[kc][:, 128 * mc:128 * (mc + 1)],
                        rhs=x_t[kc][:, pco:pco + pcn],
                        start=(kc == 0), stop=False,
                    )
                # masked bias as a rank-1 update: psum += b_v ⊗ mask
                nc.tensor.matmul(
                    ps[:, :pcn],
                    lhsT=bvrow_t[:, 128 * cti:128 * (cti + 1)],
                    rhs=maskr_t[:, pco:pco + pcn],
                    start=False, stop=True,
                )
                r0, nr = pco // W, (pcn + W - 1) // W
                v70 = v_sb[cti][:].rearrange("p (r c) -> p r c", c=VROW)
                dst = v70[:, r0:r0 + nr, 3:3 + W]
                if (pco // 512) % 2 == 0:
                    nc.scalar.activation(dst, ps[:, :pcn],
                                         mybir.ActivationFunctionType.Copy)
                else:
                    nc.vector.tensor_copy(dst, ps[:, :pcn])
            else:  # vT
                a, g = arg
                ps = od_pool_.tile([128, 512], F32, tag="od")
                for jj in range(2):
                    j = 2 * g + jj
                    pxo = PXOFF + NA * a + 128 * j
                    for kc in range(2):
                        nc.tensor.matmul(
                            ps[:, 256 * jj:256 * (jj + 1)],
                            lhsT=x_t[kc][:, pxo:pxo + 128],
                            rhs=wq_t[kc][:, 2 * C:3 * C],
                            start=(kc == 0), stop=(kc == 1),
                        )
                dst = vt_sb[a][:, 512 * g:512 * (g + 1)]
                if g % 2 == 0:
                    nc.scalar.activation(dst, ps[:, 0:512],
                                         mybir.ActivationFunctionType.Copy)
                else:
                    nc.vector.tensor_copy(dst, ps[:, 0:512])

        def v4_copies(ci):
            """DMA-replicate shifted v windows for conv chunk ci into a v4
            tile: section (pat, b) holds, at partitions 32t..32t+32, block
            b's channels shifted for tap-row dy_t; returns the tile."""
            cti, c = ci // 4, ci % 4
            v4 = v4_pool.tile([128, NSEC * SEC], BF16, tag="v4",
                              name=f"v4_{ci}")
            for pi, pat in enumerate(PATTERNS):
                for b in range(4):
                    sec = (pi * 4 + b) * SEC
                    for t, dy in enumerate(pat):
                        src = (HALO + dy + 8 * c) * VROW
                        nc.sync.dma_start(
                            v4[32 * t:32 * (t + 1), sec:sec + SEC],
                            v_sb[cti][32 * b:32 * (b + 1), src:src + SEC])
            return v4

        def conv_chunk_mms(ci, v4, ps, lo, hi):
            cti, c = ci // 4, ci % 4
            for mi in range(lo, hi):
                dxi, rem = divmod(mi, 2 * 4)
                pi, b = divmod(rem, 4)
                pat = PATTERNS[pi]
                kp = 32 * len(pat)
                sec = (pi * 4 + b) * SEC
                v4r = v4[0:kp, sec:sec + SEC].rearrange(
                    "p (r c) -> p r c", c=VROW)
                wcol = 32 * (((cti * 2 + pi) * 7 + dxi) * 4 + b)
                nc.tensor.matmul(
                    ps[32 * b:32 * (b + 1), :],
                    lhsT=w4_t[0:kp, wcol:wcol + 32],
                    rhs=v4r[:, 0:8, dxi:dxi + W],
                    start=(dxi == 0 and pi == 0),
                    stop=(dxi == 6 and pi == len(PATTERNS) - 1),
                    skip_group_check=True,
                    tile_position=(0, 32 * b),
                )

        def conv_chunk_evac(ci, ps):
            # pin = conv_psum + attention-output chunk (same 512 cols)
            cti, c = ci // 4, ci % 4
            nc.vector.tensor_tensor(
                pin_sb[cti][:, 512 * c:512 * (c + 1)], ps,
                onorm_sb[cti][:, 512 * c:512 * (c + 1)],
                mybir.AluOpType.add)

        # PE filler schedule: vT0+v0 early (conv chunk 0 copies need v0 at
        # i0), v1/vT1/Q1K1 spread over i1-i3 (Q1/K1 first read at i4).
        tail_work = (
            [("vT", (0, g)) for g in range(4)]
            + [("v", (4, 512 * t)) for t in range(5)]
            + [("vT", (1, g)) for g in range(4)]
            + [("v", (5, 512 * t)) for t in range(5)]
            + [("qk", (1, 512 * t)) for t in range(4)]
            + [("qk", (3, 512 * t)) for t in range(4)]
        )
        tail_sched = {0: 9, 1: 8, 2: 5, 3: 4}
        # v4 replication DMAs prefetched 2-3 iters ahead of consumption
        v4_sched = {0: [0, 1, 2], 1: [3], 3: [4, 5], 4: [6], 5: [7]}

        def emit_proj(pc):
            for mc in range(2):
                ps = od_pool.tile([128, 512], F32, tag="od")
                for kc in range(2):
                    nc.tensor.matmul(
                        ps[:],
                        lhsT=wp_t[kc][:, 128 * mc:128 * (mc + 1)],
                        rhs=pin_sb[kc][:, 512 * pc:512 * (pc + 1)],
                        start=(kc == 0), stop=(kc == 1),
                    )
                nc.scalar.activation(
                    out_sb[mc][:, 512 * pc:512 * (pc + 1)], ps[:],
                    mybir.ActivationFunctionType.Identity,
                    bias=btot_t[mc][:, 0:1])
                nc.sync.dma_start(
                    out_ext[128 * mc:128 * (mc + 1), 512 * pc:512 * (pc + 1)],
                    out_sb[mc][:, 512 * pc:512 * (pc + 1)])

        v4_tiles = {}
        its = [(hs, a, nu) for hs in range(2) for a in range(2) for nu in range(2)]
        for inum, (hs, a, nu) in enumerate(its):
            no = NA * a + 512 * nu   # n offset in core px
            units = [(j, hp) for j in range(8) for hp in range(4)]
            # ---- phase A: S + exp -> E tiles, with PE filler interleaved
            # between groups so exp latency on the sA/sB chains never
            # stalls the PE (which would de-ramp its p-state clock).
            pieces = []
            nwork = tail_sched.get(inum, 0)
            for _ in range(nwork):
                if tail_work:
                    pieces.append(tail_work.pop(0))
            cv_ps = None
            if inum > 0:
                ci = inum - 1
                cv_w = od_pool.tile([128, 512], F32, tag="od",
                                    name=f"cv{ci}")
                cv_ps = cv_w[:]

            def filler(slot):
                # slot 0: tail pieces; 1-3: conv thirds; 2: also v4 copies
                if slot == 0:
                    for p in pieces:
                        qkv_tail_piece(od_pool, *p)
                    return
                if slot == 2:
                    for cc in v4_sched.get(inum, []):
                        v4_tiles[cc] = v4_copies(cc)
                if inum > 0:
                    ci = inum - 1
                    lo = [0, 0, 19, 38][slot]
                    hi = [0, 19, 38, 56][slot]
                    conv_chunk_mms(ci, v4_tiles[ci], cv_ps, lo, hi)

            e_tiles = []
            ui = 0
            for gidx, gsz in enumerate(GROUP_SIZES):
                grp = units[ui:ui + gsz]
                ui += gsz
                on_dve = (gsz == 2)
                s_ps = s_ps_pool.tile([128, 512 * gsz], F32,
                                      tag="sA" if on_dve else "sB")
                for idx, (j, hp) in enumerate(grp):
                    nc.tensor.matmul(
                        s_ps[:, 512 * idx:512 * (idx + 1)],
                        lhsT=k_sb[hs][32 * hp:32 * (hp + 1),
                                      NA * a + 128 * j:NA * a + 128 * (j + 1)],
                        rhs=q_sb[hs][32 * hp:32 * (hp + 1), no:no + 512],
                        start=True, stop=True,
                        tile_position=(32 * hp, 0),
                    )
                ncols = 512 * gsz
                e_t = e_pool.tile([128, ncols], BF16,
                                  tag="ed" if on_dve else "e",
                                  bufs=7 if on_dve else 6)
                if on_dve:
                    nc.vector.tensor_scalar(
                        e_t[:, :ncols].bitcast(I16), s_ps[:, :ncols],
                        EXP_C1, EXP_C2,
                        mybir.AluOpType.mult, mybir.AluOpType.add)
                else:
                    nc.scalar.activation(
                        e_t[:, :ncols], s_ps[:, :ncols],
                        mybir.ActivationFunctionType.Exp)
                e_tiles.append((grp, e_t))
                if gidx == 1:
                    filler(0)
                elif gidx in (3, 7, 11):
                    filler({3: 1, 7: 2, 11: 3}[gidx])
            if inum > 0:
                conv_chunk_evac(inum - 1, cv_ps)
                del v4_tiles[inum - 1]
                # proj chunk pc as soon as pin[1][pc] completes (evac ci=4+pc)
                if inum >= 5:
                    emit_proj(inum - 5)
            # ---- phase B: dense O + den burst ----
            o_ps = od_pool.tile([128, 512], F32, tag="od")
            den_ps = od_pool.tile([128, 512], F32, tag="od")
            for grp, e_t in e_tiles:
                for idx, (j, hp) in enumerate(grp):
                    first, last = (j == 0), (j == 7)
                    nc.tensor.matmul(
                        o_ps[32 * hp:32 * (hp + 1), :],
                        lhsT=vt_sb[a][:, 256 * j + 32 * (4 * hs + hp):
                                       256 * j + 32 * (4 * hs + hp + 1)],
                        rhs=e_t[:, 512 * idx:512 * (idx + 1)],
                        start=first, stop=last,
                        skip_group_check=True,
                        tile_position=(0, 32 * hp),
                    )
                    nc.tensor.matmul(
                        den_ps[32 * hp:32 * (hp + 1), :],
                        lhsT=ones_t[:, 0:32],
                        rhs=e_t[:, 512 * idx:512 * (idx + 1)],
                        start=first, stop=last,
                        skip_group_check=True,
                        tile_position=(0, 32 * hp),
                    )
            if inum == 7:
                # chunk 7's matmuls run while the last exps/O drain; its
                # evacuation (needs onorm chunk 7) happens in the tail.
                ps7w = od_pool.tile([128, 512], F32, tag="od", name="cv7")
                ps7 = ps7w[:]
                conv_chunk_mms(7, v4_tiles[7], ps7, 0, 56)
                del v4_tiles[7]
            rd32 = wk_pool.tile([128, 512], F32, tag="rd32")
            nc.vector.reciprocal_approx_fast(rd32[:], den_ps[:])
            nc.vector.tensor_mul(
                onorm_sb[hs][:, no:no + 512], o_ps[:], rd32[:])

        # ---------------- tail: last conv chunk + proj ----------------
        conv_chunk_evac(7, ps7)
        emit_proj(3)

        od_ctx.__exit__(None, None, None)
        s_ps_ctx.__exit__(None, None, None)

    nc.finalize()
    return nc


_GRAPH = None


def kernel(**inputs):
    global _GRAPH
    inputs = {k: np.asarray(v, np.float32) for k, v in inputs.items()}
    x = inputs["x"]

    def fold(g, b, m, v):
        inv = g / np.sqrt(v + EPS)
        return inv, b - m * inv

    sq, bq = fold(inputs["qkv_g"], inputs["qkv_b"], inputs["qkv_m"], inputs["qkv_v"])
    spe, bpe = fold(inputs["pe_g"], inputs["pe_b"], inputs["pe_m"], inputs["pe_v"])
    sp, bp = fold(inputs["proj_g"], inputs["proj_b"], inputs["proj_m"], inputs["proj_v"])

    wqkv = np.asarray(inputs["qkv_w"], np.float32)[:, :, 0, 0] * sq[:, None]  # [768,256]
    bqkv = np.asarray(bq, np.float32)
    # permute rows to head-major [Q(256); K(256); V(256)]
    perm = np.empty(3 * C, np.int64)
    for h in range(NH):
        for t in range(3):
            for d in range(HD):
                perm[t * C + HD * h + d] = 3 * HD * h + HD * t + d
    wqkv = wqkv[perm]
    bqkv = bqkv[perm]
    scale = HD ** -0.5
    wqkv[:C] *= scale
    bqkv[:C] *= scale
    b_v = bqkv[2 * C:].copy()

    wpe = np.asarray(inputs["pe_w"], np.float32)[:, 0].reshape(C, 49) * spe[:, None]
    wproj = np.asarray(inputs["proj_w"], np.float32)[:, :, 0, 0] * sp[:, None]
    btot = bp + wproj @ (b_v + bpe)

    wqkv_T = np.ascontiguousarray(wqkv.T).astype(BF16NP)          # [256, 768]
    wproj_T = np.ascontiguousarray(wproj.T).astype(BF16NP)        # [256, 256]
    bqkv_c = np.ascontiguousarray(bqkv[:, None]).astype(np.float32)
    btot_c = np.ascontiguousarray(btot[:, None]).astype(np.float32)

    # packed conv weights: w4[32t+cc, col(cti,pi,dxi,b)*32+m] =
    #   delta(cc,m) * wpe[128*cti+32*b+m, tap(dy_t, dx)]
    w4 = np.zeros((128, 2 * 2 * 7 * 4 * 32), np.float32)
    for cti in range(2):
        for pi, pat in enumerate(PATTERNS):
            for dxi in range(7):
                dx = dxi - 3
                for b in range(4):
                    col = 32 * (((cti * 2 + pi) * 7 + dxi) * 4 + b)
                    for t, dy in enumerate(pat):
                        tap = (dy + 3) * 7 + (dx + 3)
                        ch0 = 128 * cti + 32 * b
                        for m in range(32):
                            w4[32 * t + m, col + m] = wpe[ch0 + m, tap]
    w4 = np.ascontiguousarray(w4).astype(BF16NP)

    xp = np.zeros((B, C, H + 2 * HALO, W), np.float32)
    xp[:, :, HALO:HALO + H] = x

    in_maps = []
    for i in range(NCORES):
        b, r0 = i // 2, 32 * (i % 2)
        slab = xp[b, :, r0:r0 + SLAB_ROWS, :].reshape(C, PX)
        vmask = np.zeros((1, PX), np.float32)
        vr = np.zeros(SLAB_ROWS, np.float32)
        if i % 2 == 0:
            vr[HALO:] = 1.0          # slab rows 0-2 are outside the image
        else:
            vr[:SLAB_ROWS - HALO] = 1.0
        vmask[0] = np.repeat(vr, W)
        in_maps.append({
            "x": slab.astype(BF16NP),
            "vmask": vmask.astype(BF16NP),
            "bvrow": np.ascontiguousarray(b_v[None, :]).astype(BF16NP),
            "w4": w4,
            "wqkv": wqkv_T,
            "bqkv": bqkv_c,
            "wproj": wproj_T,
            "btot": btot_c,
        })

    if _GRAPH is None:
        _GRAPH = _build_graph()

    trace = os.environ.get("BASS_KERNEL_TRACE") == "1"
    res = run_bass_kernel_spmd(_GRAPH, in_maps, list(range(NCORES)), trace=trace)
    LAST_EXEC_NS[0] = res.exec_time_ns
    LAST_RESULTS[0] = res.results[0]

    out = np.empty((B, C, H, W), np.float32)
    for i in range(NCORES):
        b, r0 = i // 2, 32 * (i % 2)
        out[b, :, r0:r0 + 32, :] = np.asarray(
            res.results[i]["out"], np.float32).reshape(C, 32, W)
    return out
